# revision 2
# baseline (speedup 1.0000x reference)
"""Trainium2 Bass kernel for nn_CanonicalMicrocircuit (gnn_message_passing).

Math note: the reference module starts from all-zero recurrent state and only
returns `all_out * (1 - g)`, so every einsum against the zero state vanishes,
the inhibitory population and the inter-column lateral tensor are dead code,
and only layer 0 of the excitatory update survives:

    x0_c  = relu((1-exp(-1/tau_c)) * (blat_e[c,0] + bfb_e[c,0]) - thr_c)
    x0_c /= (||x0_c|| + 1e-8)
    out_c = relu(Wexc[c,0] @ x0_c + bexc[c,0])            # [H] per column
    h     = sum_c Wg1[:, cH:(c+1)H] @ out_c + bg1         # [H]
    r     = relu(h)
    g_c   = sigmoid(Wg2[cH:(c+1)H, :] @ r + bg2[cH:(c+1)H])
    final_c = out_c * (1 - g_c)                           # concat -> [C*H]

Sharding: one column per NeuronCore (C == 8 == n_cores).  Each core holds its
column's Wexc slice plus the matching column-block of Wg1 and row-block of
Wg2.  The only communication is one 4 KB AllGather of the per-core Wg1
partial products, summed locally on every core.

Matvec strategy: the PE computes out = lhsT.T @ rhs with lhsT stationary.  We
keep the *vector* stationary (M=1, trivial weight load) and stream the weight
matrix as the moving operand at N elems/cycle.  That requires W^T in SBUF, so
the host pre-transposes each core's weight shards during input prep (layout
only -- device still reads the same 12 MB/core from HBM, which is the
bottleneck in this memory-bound regime).
"""

import numpy as np

import concourse.bass as bass
import concourse.bacc as bacc
import concourse.mybir as mybir
import concourse.tile as tile
from concourse.bass_utils import run_bass_kernel_spmd

C = 8
F = 512
L = 4
H = 1024
HI = 256
NCORES = 8
P = 128
KT = H // P  # 8 k-tiles per 1024-long contraction
FP = mybir.dt.float32

_CACHE = {}


def _build_nc():
    nc = bacc.Bacc(
        "TRN2",
        target_bir_lowering=False,
        debug=False,
        enable_asserts=False,
        num_devices=NCORES,
    )

    w1t = nc.dram_tensor("w1t", [H, H], FP, kind="ExternalInput")  # Wexc[c,0].T
    w2t = nc.dram_tensor("w2t", [H, H], FP, kind="ExternalInput")  # Wg1[:,blk].T
    w3t = nc.dram_tensor("w3t", [H, H], FP, kind="ExternalInput")  # Wg2[blk,:].T
    vecs = nc.dram_tensor("vecs", [5, H], FP, kind="ExternalInput")
    # rows: 0=blat_e[c,0], 1=bfb_e[c,0], 2=bexc[c,0], 3=bg1, 4=bg2[blk]
    scal = nc.dram_tensor("scal", [1, 2], FP, kind="ExternalInput")  # tau, thr
    fin = nc.dram_tensor("final", [1, H], FP, kind="ExternalOutput")

    AF = mybir.ActivationFunctionType
    ALU = mybir.AluOpType

    with tile.TileContext(nc) as tc:
        with (
            tc.tile_pool(name="sb", bufs=1) as sb,
            tc.tile_pool(name="ps_row", bufs=4, space="PSUM") as ps_row,
            tc.tile_pool(name="ps_tp", bufs=2, space="PSUM") as ps_tp,
            tc.tile_pool(name="ps_misc", bufs=1, space="PSUM") as ps_misc,
            tc.tile_pool(name="dram", bufs=1, space="DRAM") as dram,
        ):
            # ---- constants ----
            ones_r = sb.tile([1, P], FP, tag="ones_r")   # row of ones (bcast lhsT)
            ones_c = sb.tile([P, 1], FP, tag="ones_c")   # column of ones
            ones_8 = sb.tile([KT, 1], FP, tag="ones_8")  # 8-partition ones
            one_11 = sb.tile([1, 1], FP, tag="one_11")
            nc.vector.memset(ones_r[:], 1.0)
            nc.vector.memset(ones_c[:], 1.0)
            nc.vector.memset(ones_8[:], 1.0)
            nc.vector.memset(one_11[:], 1.0)

            # ---- small input loads (ACT hwdge ring; keep SP ring for weights)
            blat_t = sb.tile([P, KT], FP, tag="blat")
            bfb_t = sb.tile([P, KT], FP, tag="bfb")
            nc.scalar.dma_start(blat_t[:], vecs.ap()[0].rearrange("(t p) -> p t", p=P))
            nc.scalar.dma_start(bfb_t[:], vecs.ap()[1].rearrange("(t p) -> p t", p=P))
            bexc_row = sb.tile([1, H], FP, tag="bexc")
            bg1_row = sb.tile([1, H], FP, tag="bg1")
            bg2_row = sb.tile([1, H], FP, tag="bg2")
            nc.scalar.dma_start(bexc_row[:], vecs.ap()[2:3])
            nc.scalar.dma_start(bg1_row[:], vecs.ap()[3:4])
            nc.scalar.dma_start(bg2_row[:], vecs.ap()[4:5])
            scal_t = sb.tile([1, 2], FP, tag="scal")
            nc.scalar.dma_start(scal_t[:], scal.ap())

            # ---- weight loads (SP hwdge ring, program order = FIFO) ----
            def load_wT(name, dram_t):
                w = sb.tile([P, KT * H], FP, tag=name)
                src = dram_t.ap().rearrange("(k p) i -> p k i", p=P)
                dst = w[:].rearrange("p (k i) -> p k i", k=KT)
                nc.sync.dma_start(dst, src)
                return w

            w1 = load_wT("w1", w1t)
            w2 = load_wT("w2", w2t)
            w3 = load_wT("w3", w3t)

            # ---- per-column scalars: a1 = 1-exp(-1/tau); nthr = -thr ----
            rt = sb.tile([1, 1], FP, tag="rt")
            nc.vector.reciprocal(rt[:], scal_t[0:1, 0:1])
            ea = sb.tile([1, 1], FP, tag="ea")
            nc.scalar.activation(ea[:], rt[:], AF.Exp, scale=-1.0)  # exp(-1/tau)
            oma = sb.tile([1, 1], FP, tag="oma")
            nc.scalar.activation(oma[:], ea[:], AF.Copy, scale=-1.0, bias=1.0)
            nthr = sb.tile([1, 1], FP, tag="nthr")
            nc.scalar.activation(nthr[:], scal_t[0:1, 1:2], AF.Copy, scale=-1.0)

            # broadcast the two scalars to all 128 partitions via PE
            ps_b = ps_misc.tile([P, 2], FP, tag="misc")
            nc.tensor.matmul(ps_b[:, 0:1], ones_r[:], oma[:], start=True, stop=True)
            nc.tensor.matmul(ps_b[:, 1:2], ones_r[:], nthr[:], start=True, stop=True)
            bvec = sb.tile([P, 2], FP, tag="bvec")
            nc.vector.tensor_copy(bvec[:], ps_b[:])

            # ---- x0 = normalize(relu((1-a)*(blat+bfb) - thr)) in [128,8] ----
            tot = sb.tile([P, KT], FP, tag="tot")
            nc.vector.tensor_add(tot[:], blat_t[:], bfb_t[:])
            xr = sb.tile([P, KT], FP, tag="xr")
            nc.vector.tensor_scalar(
                xr[:], tot[:], bvec[:, 0:1], bvec[:, 1:2], op0=ALU.mult, op1=ALU.add
            )
            nc.vector.tensor_scalar_max(xr[:], xr[:], 0.0)
            sq = sb.tile([P, KT], FP, tag="sq")
            nc.vector.tensor_mul(sq[:], xr[:], xr[:])
            ssum = sb.tile([P, 1], FP, tag="ssum")
            nc.vector.tensor_reduce(
                ssum[:], sq[:], axis=mybir.AxisListType.X, op=ALU.add
            )
            ps_n = ps_misc.tile([1, 1], FP, tag="misc")
            nc.tensor.matmul(ps_n[:], ssum[:], ones_c[:], start=True, stop=True)
            nrm = sb.tile([1, 1], FP, tag="nrm")
            nc.scalar.activation(nrm[:], ps_n[:], AF.Sqrt)
            nc.scalar.activation(nrm[:], nrm[:], AF.Copy, bias=1e-8)
            inv = sb.tile([1, 1], FP, tag="inv")
            nc.vector.reciprocal(inv[:], nrm[:])
            ps_i = ps_misc.tile([P, 1], FP, tag="misc")
            nc.tensor.matmul(ps_i[:], ones_r[:], inv[:], start=True, stop=True)
            invb = sb.tile([P, 1], FP, tag="invb")
            nc.vector.tensor_copy(invb[:], ps_i[:])
            xn = sb.tile([P, KT], FP, tag="xn")
            nc.vector.tensor_scalar_mul(xn[:], xr[:], invb[:, 0:1])

            # ---- helper: row_out[1,H] = act(W^T.T @ vec + bias_row) ----
            def matvec_rows(w, vec_col, bias_row, func, scale=1.0, tag="mv"):
                """vec_col: [128, KT] sbuf; returns [1, H] sbuf row."""
                row = sb.tile([1, H], FP, tag=tag)
                for hhalf in range(2):
                    ps = ps_row.tile([1, 512], FP, tag="row")
                    for k in range(KT):
                        nc.tensor.matmul(
                            ps[:],
                            vec_col[:, k : k + 1],
                            w[:, k * H + hhalf * 512 : k * H + hhalf * 512 + 512],
                            start=(k == 0),
                            stop=(k == KT - 1 and bias_row is None),
                        )
                    if bias_row is not None:
                        nc.tensor.matmul(
                            ps[:],
                            one_11[:],
                            bias_row[0:1, hhalf * 512 : hhalf * 512 + 512],
                            start=False,
                            stop=True,
                        )
                    nc.scalar.activation(
                        row[0:1, hhalf * 512 : hhalf * 512 + 512],
                        ps[:],
                        func,
                        scale=scale,
                    )
                return row

            def row_to_col(row, tag):
                """[1, H] row -> [128, KT] partition layout (vec[t*128+p])."""
                ps = ps_tp.tile([P, KT], FP, tag="tp")
                for t in range(KT):
                    nc.tensor.matmul(
                        ps[:, t : t + 1],
                        row[0:1, t * P : (t + 1) * P],
                        one_11[:],
                        start=True,
                        stop=True,
                    )
                col = sb.tile([P, KT], FP, tag=tag)
                nc.vector.tensor_copy(col[:], ps[:])
                return col

            # Stage A: out_c = relu(Wexc @ x0 + bexc)
            outa_row = matvec_rows(w1, xn, bexc_row, AF.Relu, tag="outa")
            outa_col = row_to_col(outa_row, "outa_col")

            # Stage B: partial h = Wg1_blk @ out_c  (bias added after gather)
            hp_row = matvec_rows(w2, outa_col, None, AF.Copy, tag="hp")

            # AllGather partials (4 KB per rank) and sum locally
            cc_in = dram.tile([1, H], FP, tag="cc_in")
            cc_out = dram.tile([NCORES, H], FP, tag="cc_out")
            nc.scalar.dma_start(cc_in[:], hp_row[:])
            nc.gpsimd.collective_compute(
                "AllGather",
                ALU.bypass,
                replica_groups=[list(range(NCORES))],
                ins=[cc_in[:]],
                outs=[cc_out[:]],
            )
            agt = sb.tile([NCORES, H], FP, tag="agt")
            nc.scalar.dma_start(agt[:], cc_out[:])

            r_row = sb.tile([1, H], FP, tag="r_row")
            for hhalf in range(2):
                ps = ps_row.tile([1, 512], FP, tag="row")
                nc.tensor.matmul(
                    ps[:],
                    ones_8[:],
                    agt[:, hhalf * 512 : hhalf * 512 + 512],
                    start=True,
                    stop=False,
                )
                nc.tensor.matmul(
                    ps[:],
                    one_11[:],
                    bg1_row[0:1, hhalf * 512 : hhalf * 512 + 512],
                    start=False,
                    stop=True,
                )
                nc.scalar.activation(
                    r_row[0:1, hhalf * 512 : hhalf * 512 + 512], ps[:], AF.Relu
                )
            r_col = row_to_col(r_row, "r_col")

            # Stage C: s = sigmoid(-(Wg2_blk @ r + bg2)) = 1 - g
            s_row = matvec_rows(w3, r_col, bg2_row, AF.Sigmoid, scale=-1.0, tag="s")

            # final_c = out_c * s
            fin_row = sb.tile([1, H], FP, tag="fin")
            nc.vector.tensor_mul(fin_row[:], outa_row[:], s_row[:])
            nc.sync.dma_start(fin.ap(), fin_row[:])

    nc.compile()
    return nc


def get_nc():
    if "nc" not in _CACHE:
        _CACHE["nc"] = _build_nc()
    return _CACHE["nc"]


def make_in_maps(inputs):
    """Slice + pre-transpose the full inputs into 8 per-core input dicts."""
    Wexc = np.asarray(inputs["Wexc"], dtype=np.float32)
    Wg1 = np.asarray(inputs["Wg1"], dtype=np.float32)
    Wg2 = np.asarray(inputs["Wg2"], dtype=np.float32)
    blat = np.asarray(inputs["blat_e"], dtype=np.float32)
    bfb = np.asarray(inputs["bfb_e"], dtype=np.float32)
    bexc = np.asarray(inputs["bexc"], dtype=np.float32)
    bg1 = np.asarray(inputs["bg1"], dtype=np.float32)
    bg2 = np.asarray(inputs["bg2"], dtype=np.float32)
    tau = np.asarray(inputs["tau_exc"], dtype=np.float32)
    thr = np.asarray(inputs["threshold"], dtype=np.float32)

    in_maps = []
    for c in range(NCORES):
        sl = slice(c * H, (c + 1) * H)
        vecs = np.stack([blat[c, 0], bfb[c, 0], bexc[c, 0], bg1, bg2[sl]])
        in_maps.append(
            {
                "w1t": np.ascontiguousarray(Wexc[c, 0].T),
                "w2t": np.ascontiguousarray(Wg1[:, sl].T),
                "w3t": np.ascontiguousarray(Wg2[sl, :].T),
                "vecs": np.ascontiguousarray(vecs),
                "scal": np.array([[tau[c], thr[c]]], dtype=np.float32),
            }
        )
    return in_maps


def kernel(**inputs):
    nc = get_nc()
    in_maps = make_in_maps(inputs)
    res = run_bass_kernel_spmd(nc, in_maps, core_ids=list(range(NCORES)))
    _CACHE["last_result"] = res
    out = np.concatenate(
        [res.results[c]["final"].reshape(-1) for c in range(NCORES)]
    ).astype(np.float32)
    return out


# revision 16
# speedup vs baseline: 1.3604x; 1.3604x over previous
"""Trainium2 Bass kernel for nn_CanonicalMicrocircuit (gnn_message_passing).

Math note: the reference module starts from all-zero recurrent state and only
returns `all_out * (1 - g)`, so every einsum against the zero state vanishes,
the inhibitory population and the inter-column lateral tensor are dead code,
and only layer 0 of the excitatory update survives:

    x0_c  = relu((1-exp(-1/tau_c)) * (blat_e[c,0] + bfb_e[c,0]) - thr_c)
    x0_c /= (||x0_c|| + 1e-8)
    out_c = relu(Wexc[c,0] @ x0_c + bexc[c,0])            # [H] per column
    h     = sum_c Wg1[:, cH:(c+1)H] @ out_c + bg1         # [H]
    r     = relu(h)
    g_c   = sigmoid(Wg2[cH:(c+1)H, :] @ r + bg2[cH:(c+1)H])
    final_c = out_c * (1 - g_c)                           # concat -> [C*H]

Sharding: one column per NeuronCore (C == 8 == n_cores).  Each core holds its
column's Wexc slice plus the matching column-block of Wg1 and row-block of
Wg2.  The only communication is one 4 KB AllGather of the per-core Wg1
partial products, summed locally on every core.

Engine plan (from profiling): the runtime inserts a collective-init barrier
on the CC stream at kernel entry (~46 us here) that also gates the Tensor
queue, and the ncfw AllGather costs ~39 us after trigger.  So stages A and B
run on DVE+GpSimd (scalar_tensor_tensor with accum_out = per-row dot
products against partition-broadcast vectors), pipelined behind the weight
DMAs and finishing before the barrier clears; the AllGather triggers as
early as its input exists; stage C (post-AllGather) is split between the PE
(rows 0-511, host-pre-transposed shard) and DVE (rows 512-1023, natural
shard) to shorten the tail.
"""

import numpy as np

import concourse.bass as bass
import concourse.bacc as bacc
import concourse.mybir as mybir
import concourse.tile as tile
from concourse.bass_utils import run_bass_kernel_spmd

C = 8
F = 512
L = 4
H = 1024
HI = 256
NCORES = 8
P = 128
KT = H // P  # 8 row/k tiles per 1024 dim
FP = mybir.dt.float32
TOP = 384  # stage-C rows on the PE
BOT = H - TOP
KB = BOT // P  # 5 DVE row-tiles in stage C

_CACHE = {}


def _build_nc():
    nc = bacc.Bacc(
        "TRN2",
        target_bir_lowering=False,
        debug=False,
        enable_asserts=False,
        num_devices=NCORES,
    )

    w1 = nc.dram_tensor("w1", [H, H], FP, kind="ExternalInput")  # Wexc[c,0] natural
    w2 = nc.dram_tensor("w2", [H, H], FP, kind="ExternalInput")  # Wg1[:,blk] natural
    w3t = nc.dram_tensor("w3t", [H, TOP], FP, kind="ExternalInput")  # top.T
    w3n = nc.dram_tensor("w3n", [BOT, H], FP, kind="ExternalInput")  # bottom nat
    vecs = nc.dram_tensor("vecs", [6, H], FP, kind="ExternalInput")
    eye = nc.dram_tensor("eye", [P, P], FP, kind="ExternalInput")
    # rows (rho = p-major storage permutation, see make_in_maps):
    # 0=blat, 1=bfb, 2=bexc[rho], 3=bg1[rho], 4=[bg2p[:512], bg2p-bot-col], 5=[tau, thr]
    fin = nc.dram_tensor("final", [1, H], FP, kind="ExternalOutput")

    AF = mybir.ActivationFunctionType
    ALU = mybir.AluOpType

    with tile.TileContext(nc) as tc:
        with (
            tc.tile_pool(name="sb", bufs=1) as sb,
            tc.tile_pool(name="jk", bufs=2) as jk,
            tc.tile_pool(name="ps_row", bufs=3, space="PSUM") as ps_row,
            tc.tile_pool(name="ps_tp", bufs=1, space="PSUM") as ps_tp,
            tc.tile_pool(name="dram", bufs=1, space="DRAM") as dram,
        ):
            # ---- weight loads: SP hwdge ring, FIFO in program order ----
            # W1 and W2 as 2x 2MB chunks (4 row-tiles each) for pipelining.
            def load_nat_pairs(name, dram_t):
                tiles = []
                for a in range(KT // 4):
                    t = sb.tile([P, 4, H], FP, tag=f"{name}{a}")
                    src = dram_t.ap()[4 * a * P : 4 * (a + 1) * P, :].rearrange(
                        "(t p) i -> p t i", p=P
                    )
                    nc.sync.dma_start(t[:], src)
                    tiles.append(t)
                return tiles  # tiles[a][:, b, :] is row-tile 4a+b

            w1_t = load_nat_pairs("w1", w1)
            w2_t = load_nat_pairs("w2", w2)
            w3t_t = sb.tile([P, KT, TOP], FP, tag="w3t")
            nc.sync.dma_start(w3t_t[:], w3t.ap().rearrange("(k p) i -> p k i", p=P))
            w3n_t = sb.tile([P, KB, H], FP, tag="w3n")
            nc.sync.dma_start(w3n_t[:], w3n.ap().rearrange("(t p) i -> p t i", p=P))

            # ---- small loads on the ACT hwdge ring ----
            vt = sb.tile([1, 6 * H], FP, tag="vecs")
            nc.scalar.dma_start(
                vt[:], vecs.ap().rearrange("a b -> (a b)").rearrange("(x n) -> x n", x=1)
            )
            bexc_col = sb.tile([P, KT], FP, tag="bexc_col")
            nc.scalar.dma_start(
                bexc_col[:], vecs.ap()[2].rearrange("(p t) -> p t", p=P)
            )
            bg2_bot = sb.tile([P, KB], FP, tag="bg2_bot")
            nc.scalar.dma_start(
                bg2_bot[:], vecs.ap()[4][TOP:H].rearrange("(p t) -> p t", p=P)
            )

            # ---- constants for the PE (post-collective stages) ----
            eye_t = sb.tile([P, P], FP, tag="eye")
            nc.scalar.dma_start(eye_t[:], eye.ap())
            ones_8 = sb.tile([KT, 1], FP, tag="ones_8")
            one_11 = sb.tile([1, 1], FP, tag="one_11")
            nc.vector.memset(ones_8[:], 1.0)
            nc.vector.memset(one_11[:], 1.0)

            # ---- x0 in row form on partition 0 ----
            rt = sb.tile([1, 1], FP, tag="rt")
            nc.vector.reciprocal(rt[:], vt[0:1, 5 * H : 5 * H + 1])
            ea = sb.tile([1, 1], FP, tag="ea")
            nc.scalar.activation(ea[:], rt[:], AF.Exp, scale=-1.0)  # exp(-1/tau)
            oma = sb.tile([1, 1], FP, tag="oma")
            nc.scalar.activation(oma[:], ea[:], AF.Copy, scale=-1.0, bias=1.0)
            nthr = sb.tile([1, 1], FP, tag="nthr")
            nc.scalar.activation(nthr[:], vt[0:1, 5 * H + 1 : 5 * H + 2], AF.Copy, scale=-1.0)

            xr = sb.tile([1, H], FP, tag="xr")
            nc.vector.tensor_add(xr[:], vt[0:1, 0:H], vt[0:1, H : 2 * H])
            nc.vector.tensor_scalar(
                xr[:], xr[:], oma[:], nthr[:], op0=ALU.mult, op1=ALU.add
            )
            nc.vector.tensor_scalar_max(xr[:], xr[:], 0.0)
            ssq = sb.tile([1, 1], FP, tag="ssq")
            sqj = jk.tile([1, H], FP, tag="sqj")
            nc.vector.scalar_tensor_tensor(
                sqj[:], xr[:], 1.0, xr[:], op0=ALU.mult, op1=ALU.mult,
                accum_out=ssq[:],
            )
            nrm = sb.tile([1, 1], FP, tag="nrm")
            nc.scalar.activation(nrm[:], ssq[:], AF.Sqrt)
            nc.scalar.activation(nrm[:], nrm[:], AF.Copy, bias=1e-8)
            inv = sb.tile([1, 1], FP, tag="inv")
            nc.vector.reciprocal(inv[:], nrm[:])
            nc.vector.tensor_scalar_mul(xr[:], xr[:], inv[:])

            xb = sb.tile([P, H], FP, tag="xb")
            nc.gpsimd.partition_broadcast(xb[:], xr[0:1, :])

            # ---- fused row-dot matvec: acc[p, t] = sum_j W[t*128+p, j]*v[j]
            def matvec_nat(tiles, vb, acc):
                for t in range(KT):
                    w_ap = tiles[t // 4][:, t % 4, :]
                    junk = jk.tile([P, H], FP, tag="jv")
                    nc.vector.scalar_tensor_tensor(
                        junk[:], w_ap, 1.0, vb[:], op0=ALU.mult, op1=ALU.mult,
                        accum_out=acc[:, t : t + 1],
                    )

            # Stage A: out_c = relu(W1 @ x0 + bexc)
            outa = sb.tile([P, KT], FP, tag="outa")
            matvec_nat(w1_t, xb, outa)
            nc.vector.tensor_add(outa[:], outa[:], bexc_col[:])
            nc.vector.tensor_scalar_max(outa[:], outa[:], 0.0)
            outa_row = sb.tile([1, H], FP, tag="outa_row")
            nc.scalar.dma_start(outa_row[:], outa[:])
            xb2 = sb.tile([P, H], FP, tag="xb2")
            nc.gpsimd.partition_broadcast(xb2[:], outa_row[0:1, :])

            # Stage B: hp = W2 @ out_c + bg1/8 (so the gathered sum includes bg1)
            bg1_col = sb.tile([P, KT], FP, tag="bg1_col")
            nc.scalar.dma_start(
                bg1_col[:], vecs.ap()[3].rearrange("(p t) -> p t", p=P)
            )
            hp = sb.tile([P, KT], FP, tag="hp")
            matvec_nat(w2_t, xb2, hp)
            nc.vector.scalar_tensor_tensor(
                hp[:], bg1_col[:], 0.125, hp[:], op0=ALU.mult, op1=ALU.add
            )

            # AllGather the 4KB partials, triggered as soon as hp lands
            cc_in = dram.tile([1, H], FP, tag="cc_in")
            cc_out = dram.tile([NCORES, H], FP, tag="cc_out")
            nc.scalar.dma_start(cc_in[:], hp[:])
            nc.gpsimd.collective_compute(
                "AllGather",
                ALU.bypass,
                replica_groups=[list(range(NCORES))],
                ins=[cc_in[:]],
                outs=[cc_out[:]],
            )
            agt = sb.tile([NCORES, H], FP, tag="agt")
            nc.scalar.dma_start(agt[:], cc_out[:])

            # r = relu(sum_c partials + bg1) on the PE (free post-barrier)
            r_row = sb.tile([1, H], FP, tag="r_row")
            for hh in range(2):
                sl = slice(hh * 512, hh * 512 + 512)
                ps = ps_row.tile([1, 512], FP, tag="row")
                nc.tensor.matmul(ps[:], ones_8[:], agt[:, sl], start=True, stop=True)
                nc.scalar.activation(r_row[0:1, sl], ps[:], AF.Relu)

            # Stage C rows 0-511 on the PE: s = sigmoid(-(W3 @ r + bg2))
            psT = ps_tp.tile([P, KT], FP, tag="tp")
            for t in range(KT):
                nc.tensor.matmul(
                    psT[:, t : t + 1],
                    r_row[0:1, t * P : (t + 1) * P],
                    one_11[:],
                    start=True,
                    stop=True,
                )
            r_col = sb.tile([P, KT], FP, tag="r_col")
            nc.vector.tensor_copy(r_col[:], psT[:])

            s_row = sb.tile([1, H], FP, tag="s_row")
            ps = ps_row.tile([1, TOP], FP, tag="row")
            for k in range(KT):
                nc.tensor.matmul(
                    ps[:],
                    r_col[:, k : k + 1],
                    w3t_t[:, k, :],
                    start=(k == 0),
                    stop=False,
                )
            nc.tensor.matmul(
                ps[:], one_11[:], vt[0:1, 4 * H : 4 * H + TOP], start=False, stop=True
            )
            nc.scalar.activation(s_row[0:1, 0:TOP], ps[:], AF.Sigmoid, scale=-1.0)

            # Stage C rows 512-1023 on DVE/GpSimd
            xb3 = sb.tile([P, H], FP, tag="xb3")
            nc.gpsimd.partition_broadcast(xb3[:], r_row[0:1, :])
            zb = sb.tile([P, KB], FP, tag="zb")
            for t in range(KB):
                junk = jk.tile([P, H], FP, tag="jv")
                nc.vector.scalar_tensor_tensor(
                    junk[:], w3n_t[:, t, :], 1.0, xb3[:], op0=ALU.mult, op1=ALU.mult,
                    accum_out=zb[:, t : t + 1],
                )
            nc.vector.tensor_add(zb[:], zb[:], bg2_bot[:])
            # row-ize zb on the PE (identity transpose), sigmoid from PSUM
            psZ = ps_row.tile([1, BOT], FP, tag="row")
            for t in range(KB):
                nc.tensor.matmul(
                    psZ[0:1, t * P : (t + 1) * P],
                    zb[:, t : t + 1],
                    eye_t[:],
                    start=True,
                    stop=True,
                )
            nc.scalar.activation(s_row[0:1, TOP:H], psZ[:], AF.Sigmoid, scale=-1.0)

            # final = out_c * s  (rho-ordered row; host un-permutes)
            fin_a = sb.tile([1, TOP], FP, tag="fin_a")
            nc.vector.tensor_mul(fin_a[:], outa_row[0:1, 0:TOP], s_row[0:1, 0:TOP])
            nc.sync.dma_start(fin.ap()[0:1, 0:TOP], fin_a[:])
            fin_b = sb.tile([1, BOT], FP, tag="fin_b")
            nc.vector.tensor_mul(fin_b[:], outa_row[0:1, TOP:H], s_row[0:1, TOP:H])
            nc.sync.dma_start(fin.ap()[0:1, TOP:H], fin_b[:])

    nc.compile()
    return nc


def get_nc():
    if "nc" not in _CACHE:
        _CACHE["nc"] = _build_nc()
    return _CACHE["nc"]


def make_in_maps(inputs):
    """Slice the full inputs into 8 per-core input dicts (layout prep only).

    RHO is the p-major storage permutation: the device keeps the exchanged
    1024-vectors in storage order s with natural index rho[s] = (s%8)*128 +
    s//8, which makes every on-device transpose DMA contiguous.  The
    contractions are order-invariant, so we permute the matching weight
    columns / bias entries here and un-permute the final output on the host.
    """
    Wexc = np.asarray(inputs["Wexc"], dtype=np.float32)
    Wg1 = np.asarray(inputs["Wg1"], dtype=np.float32)
    Wg2 = np.asarray(inputs["Wg2"], dtype=np.float32)
    blat = np.asarray(inputs["blat_e"], dtype=np.float32)
    bfb = np.asarray(inputs["bfb_e"], dtype=np.float32)
    bexc = np.asarray(inputs["bexc"], dtype=np.float32)
    bg1 = np.asarray(inputs["bg1"], dtype=np.float32)
    bg2 = np.asarray(inputs["bg2"], dtype=np.float32)
    tau = np.asarray(inputs["tau_exc"], dtype=np.float32)
    thr = np.asarray(inputs["threshold"], dtype=np.float32)

    s_idx = np.arange(H)
    rho = (s_idx % KT) * P + s_idx // KT  # storage -> natural
    eye = np.eye(P, dtype=np.float32)

    in_maps = []
    for c in range(NCORES):
        sl = slice(c * H, (c + 1) * H)
        srow = np.zeros((H,), np.float32)
        srow[0], srow[1] = tau[c], thr[c]
        bg2p = bg2[sl][rho]
        # bottom col-form bias: row4[TOP + p*KB + t] = bg2p[TOP + t*128 + p]
        bg2_bot = bg2p[TOP:].reshape(KB, P).T.reshape(-1)
        row4 = np.concatenate([bg2p[:TOP], bg2_bot])
        vecs = np.stack([blat[c, 0], bfb[c, 0], bexc[c, 0][rho], bg1[rho], row4, srow])
        w3pp = Wg2[sl][np.ix_(rho, rho)]
        in_maps.append(
            {
                "w1": np.ascontiguousarray(Wexc[c, 0]),
                "w2": np.ascontiguousarray(Wg1[:, sl][:, rho]),
                "w3t": np.ascontiguousarray(w3pp[0:TOP, :].T),
                "w3n": np.ascontiguousarray(w3pp[TOP:, :]),
                "vecs": np.ascontiguousarray(vecs),
                "eye": eye,
            }
        )
    return in_maps


def kernel(**inputs):
    nc = get_nc()
    in_maps = make_in_maps(inputs)
    res = run_bass_kernel_spmd(nc, in_maps, core_ids=list(range(NCORES)))
    _CACHE["last_result"] = res
    chunks = []
    for c in range(NCORES):
        st = res.results[c]["final"].reshape(P, KT)  # storage s = p*KT + t
        chunks.append(np.ascontiguousarray(st.T).reshape(-1))  # natural t*P+p
    return np.concatenate(chunks).astype(np.float32)
